# revision 7
# baseline (speedup 1.0000x reference)
"""ChemConv Bass kernel for 8 TRN2 NeuronCores.

Math: the reference
    node_connection[a,f,i] = sum_n conn[a,n,f] * x[n,i]
    bond_score[a,o,f]      = sum_i node_connection[a,f,i] * pf[o,f,i]
    out[a,o] = sum_f bond_score[a,o,f]*bf[o,f,0] + sum_{f,c} bp[a,f,c]*bf[o,f,1+c]
collapses algebraically to one large matmul plus small ones:
    W[o,f,i]  = pf[o,f,i] * bf[o,f,0]
    Y[k=(f,n), o] = sum_i x[n,i] * W[o,f,i]          (tiny: 24576 x 64)
    out[a,o]  = sum_k conn2d[a,k] * Y[k,o] + sum_j bpT[j,a] * bf2[j,o]
where conn2d[a, (f,n)] = conn[a,n,f] (201 MB -> the memory-bound stream).

Sharding: atoms (dim a) row-slabs of 256 across 8 cores. Each core computes
out_T[o, a_slab] via PSUM accumulation over 192 K-chunks of 128.
conn is pre-transposed host-side to [K, a_slab] so DMA loads land with the
contraction dim on SBUF partitions (PE needs partition = K on both operands).
float32r streams the fp32 moving operand at ~1 cycle/row (vs 4 for fp32).
Y is computed on device from x^T (0.5 MB) instead of DMAing 6.3 MB.
"""

import numpy as np

import concourse.bass as bass
import concourse.tile as tile
from concourse import bacc, mybir
from concourse.bass_utils import run_bass_kernel_spmd

A = 2048
IN_DEPTH = 64
OUT_DEPTH = 64
F = 12
NCORES = 8
AS = A // NCORES          # 256 atoms per core
K = A * F                 # 24576 contraction length
KP = 128                  # K per matmul chunk (partition dim)
KC = K // KP              # 192 chunks
NBLK = A // KP            # 16 n-blocks per filter tap
KB = 2 * F                # bond-term contraction length (f,c) = 24
YG = 8                    # y chunks per PSUM bank group (8*64 = 512 = bank)

MM_DT = mybir.dt.float32r  # fp32 bits, full-rate PE streaming mode
F32 = mybir.dt.float32

_cache = {}


def _build_nc(repeat=1, B=16, bufs=3, y_dev=True, split_dma=False):
    """Build the per-core kernel.

    repeat: re-run the whole body N times (benchmark-only; deliverable uses 1)
    B: K-chunks per DMA batch (B*128*256*4 bytes per transfer)
    bufs: stream-pool buffering depth
    y_dev: compute Y on device from xT/Wr (vs DMA the precomputed 6.3 MB)
    split_dma: issue each conn batch as two half-DMAs on the two HWDGE rings
    """
    NB = KC // B
    nc = bacc.Bacc("TRN2", target_bir_lowering=False, debug=False)

    conn_t = nc.dram_tensor("conn_t", [K, AS], MM_DT, kind="ExternalInput").ap()
    bond_t = nc.dram_tensor("bond_t", [KB, AS], F32, kind="ExternalInput").ap()
    bf2 = nc.dram_tensor("bf2", [KB, OUT_DEPTH], F32, kind="ExternalInput").ap()
    if y_dev:
        xt = nc.dram_tensor("xt", [IN_DEPTH, A], F32, kind="ExternalInput").ap()
        wr = nc.dram_tensor("wr", [IN_DEPTH, F * OUT_DEPTH], F32, kind="ExternalInput").ap()
    else:
        ypack = nc.dram_tensor("ypack", [KP, KC * OUT_DEPTH], MM_DT, kind="ExternalInput").ap()
        ydram_v = ypack.rearrange("p (nb o) -> p nb o", nb=KC)
    out_t = nc.dram_tensor("out_t", [OUT_DEPTH, AS], F32, kind="ExternalOutput").ap()

    # DRAM view of conn_t with the chunk partition dim innermost:
    # [K, AS] -> [p=128, nb=KC, a=AS]
    conn_v = conn_t.rearrange("(nb p) a -> p nb a", p=KP)

    with tile.TileContext(nc) as tc:
        with (
            tc.tile_pool(name="const", bufs=1) as cpool,
            tc.tile_pool(name="ypool", bufs=2) as ypool,
            tc.tile_pool(name="stream", bufs=bufs) as spool,
            tc.tile_pool(name="psum", bufs=2, space="PSUM") as ppool,
            tc.tile_pool(name="ypsum", bufs=2, space="PSUM") as ypp,
        ):
            bond_sb = cpool.tile([KB, AS], F32)
            nc.sync.dma_start(bond_sb[:], bond_t[:])
            bf2_sb = cpool.tile([KB, OUT_DEPTH], F32)
            nc.sync.dma_start(bf2_sb[:], bf2[:])
            if y_dev:
                xt_sb = cpool.tile([IN_DEPTH, A], F32)
                nc.sync.dma_start(xt_sb[:], xt[:])
                wr_sb = cpool.tile([IN_DEPTH, F * OUT_DEPTH], F32)
                nc.sync.dma_start(wr_sb[:], wr[:])

            for rep in range(repeat):
                y_sb = ypool.tile([KP, KC * OUT_DEPTH], MM_DT, tag="y")
                y_v = y_sb.rearrange("p (nb o) -> p nb o", nb=KC)
                if y_dev:
                    # Y[kc=(f,nb)] chunk = xT[:, nb-block].T @ Wr[:, f-block]
                    yg_v = y_sb.rearrange("p (g x) -> p g x", g=KC // YG)
                    for g in range(KC // YG):
                        yps = ypp.tile([KP, YG * OUT_DEPTH], F32, tag="yps")
                        for j in range(YG):
                            kc = g * YG + j
                            f, nb = divmod(kc, NBLK)
                            nc.tensor.matmul(
                                yps[:, j * OUT_DEPTH:(j + 1) * OUT_DEPTH],
                                xt_sb[:, nb * KP:(nb + 1) * KP],
                                wr_sb[:, f * OUT_DEPTH:(f + 1) * OUT_DEPTH],
                                start=(j == 0),
                                stop=(j == YG - 1),
                            )
                        nc.vector.tensor_copy(yg_v[:, g, :], yps[:].bitcast(MM_DT))
                else:
                    for i in range(NB):
                        nc.sync.dma_start(y_v[:, i * B:(i + 1) * B, :],
                                          ydram_v[:, i * B:(i + 1) * B, :])

                acc = ppool.tile([OUT_DEPTH, AS], F32, tag="acc")

                # bond term opens the PSUM accumulation group (its inputs
                # arrive first; PE can start while conn still streams)
                nc.tensor.matmul(acc[:], bf2_sb[:], bond_sb[:], start=True, stop=False)

                for bt in range(NB):
                    ctile = spool.tile([KP, B, AS], MM_DT, tag="conn")
                    if split_dma:
                        h = B // 2
                        nc.sync.dma_start(ctile[:, :h, :],
                                          conn_v[:, bt * B:bt * B + h, :])
                        nc.scalar.dma_start(ctile[:, h:, :],
                                            conn_v[:, bt * B + h:(bt + 1) * B, :])
                    else:
                        nc.sync.dma_start(ctile[:], conn_v[:, bt * B:(bt + 1) * B, :])
                    for b in range(B):
                        kc = bt * B + b
                        nc.tensor.matmul(
                            acc[:],
                            y_v[:, kc, :],
                            ctile[:, b, :],
                            start=False,
                            stop=(kc == KC - 1),
                        )

                out_sb = spool.tile([OUT_DEPTH, AS], F32, tag="osb")
                nc.scalar.copy(out_sb[:], acc[:])
                nc.sync.dma_start(out_t[:], out_sb[:])

    nc.compile()
    return nc


def _prep(node_property_tensor, connectivity_tensor, bond_property_tensor,
          property_filters, bond_filters, y_dev=True):
    x = np.asarray(node_property_tensor, dtype=np.float32)
    conn = np.asarray(connectivity_tensor, dtype=np.float32)
    bp = np.asarray(bond_property_tensor, dtype=np.float32)
    pf = np.asarray(property_filters, dtype=np.float32)
    bf = np.asarray(bond_filters, dtype=np.float32)

    W = pf * bf[:, :, 0:1]                                # (O, F, I)
    wr = np.ascontiguousarray(W.transpose(2, 1, 0).reshape(IN_DEPTH, F * OUT_DEPTH))
    bf2 = np.ascontiguousarray(bf[:, :, 1:3].reshape(OUT_DEPTH, KB).T)  # (24, O)

    common = {"bf2": bf2}
    if y_dev:
        common["xt"] = np.ascontiguousarray(x.T)
        common["wr"] = wr
        # k = (f, n) major: conn_t[k, a] = conn[a, n, f]
        connT = np.ascontiguousarray(conn.transpose(2, 1, 0))  # (F, A_n, A_a)
        connT2 = connT.reshape(K, A)
    else:
        Y = x @ wr                                        # (A, F*O), k = n*F+f
        Y2d = Y.reshape(A * F, OUT_DEPTH)
        common["ypack"] = np.ascontiguousarray(
            Y2d.reshape(KC, KP, OUT_DEPTH).transpose(1, 0, 2)
            .reshape(KP, KC * OUT_DEPTH))
        connT2 = np.ascontiguousarray(conn.reshape(A, K).T)  # (K, A)

    in_maps = []
    for c in range(NCORES):
        sl = slice(c * AS, (c + 1) * AS)
        in_maps.append({
            "conn_t": np.ascontiguousarray(connT2[:, sl]),
            "bond_t": np.ascontiguousarray(bp[sl].reshape(AS, KB).T),
            **common,
        })
    return in_maps


def kernel(node_property_tensor, connectivity_tensor, bond_property_tensor,
           property_filters, bond_filters):
    in_maps = _prep(node_property_tensor, connectivity_tensor,
                    bond_property_tensor, property_filters, bond_filters)

    if "nc" not in _cache:
        _cache["nc"] = _build_nc()
    nc = _cache["nc"]

    res = run_bass_kernel_spmd(nc, in_maps, core_ids=list(range(NCORES)))

    out = np.empty((A, OUT_DEPTH), dtype=np.float32)
    for c in range(NCORES):
        out[c * AS:(c + 1) * AS, :] = res.results[c]["out_t"].T
    return out
